# revision 23
# baseline (speedup 1.0000x reference)
"""DenseFastGAT forward on 8 Trainium2 NeuronCores (Bass/Tile).

Math (per batch b):
  z  = x @ W.T + bW                                  [N, O]
  ai = z @ wai.T + bai ; aj = z @ waj.T + baj        [N]
  e  = leakyrelu(ai_i + aj_j, 0.2)
  att = softmax_row(where(adj>0, e, -9e15) ++ sink(-1e9))[:, :N]
  out = att @ z

Kernel strategy (v2):
  - ai/aj fold to x @ (W.T @ w.T) + const on host in f64 (tiny per-batch
    [N,256]@[256,1]), fed as exp'd vectors.
  - Sharding: 8 cores = 2 batches x 4 row-slabs of NI=1024 rows each.
    Each core gets the full-batch adjacency TRANSPOSED slab adjsT[j, i_slab]
    (bf16, exact for 0/1) and x.T (bf16) to compute z redundantly.
  - Softmax rows are invariant to any per-row scale, so divide the
    unnormalized weights by exp(ai_i) (a pure column factor in the [j,i]
    layout):  p'[j,i] = adj * max(u1_j, e2v_i*u2_j)  with
    u1 = exp(aj), u2 = exp(0.2 aj), e2v = exp(-0.8 ai).
    This makes the whole exp/leakyrelu field ONE fused 2-op
    tensor_scalar per j-tile:  (e2v_bc mult u2) max u1  -- 4x DVE mode --
    plus one quad-merged tensor_tensor mask multiply (2x mode).
  - p' is the matmul stationary operand: out[i_chunk,:] += p'[:,chunk].T @
    z_aug where z_aug = [z | ones]; the ones column yields the softmax
    denominator as output column 256 for free.
  - bW cancels out of the attention logits (folded on host) and is a pure
    additive constant on the output (att rows sum to 1), so z is computed
    WITHOUT bias on device (2 matmuls per tile instead of 3) and bW is
    added during host-side unsharding.
  - z PSUM->SBUF bf16 casts ride ScalarE; normalize (x 1/den) also rides
    ScalarE (activation Copy with per-partition scale), keeping VectorE
    for the field builds only. TensorE stays densely scheduled (z matmuls
    then attention matmuls back-to-back) to hold the warm 2.4 GHz clock.
"""

import numpy as np
import ml_dtypes

B = 2
N = 4096
IN_F = 256
O = 256
NCORES = 8
SLABS_PER_B = 4
NI = N // SLABS_PER_B        # 1024 rows per core
JT = N // 128                # 32 j-tiles
NQ = JT // 4                 # 8 quads of j-tiles
IC = NI // 128               # 8 output chunks per core
ALPHA = 0.2

_CACHE = {}


def _build():
    import concourse.bacc as bacc
    import concourse.mybir as mybir
    import concourse.tile as tile

    dt = mybir.dt
    ALU = mybir.AluOpType

    nc = bacc.Bacc("TRN2", target_bir_lowering=False, debug=False,
                   num_devices=NCORES)

    adjsT = nc.dram_tensor("adjsT", [N, NI], dt.bfloat16, kind="ExternalInput")
    xpack_d = nc.dram_tensor("xpack", [128, 3584], dt.bfloat16,
                             kind="ExternalInput")
    xrest_d = nc.dram_tensor("xrest", [IN_F, N - NI], dt.bfloat16,
                             kind="ExternalInput")
    u12_col = nc.dram_tensor("u12_col", [128, 2 * JT], dt.float32, kind="ExternalInput")
    out = nc.dram_tensor("out", [NI, O], dt.bfloat16, kind="ExternalOutput")

    adjq_view = adjsT.ap().rearrange("(q k p) i -> q p k i", k=4, p=128)

    with tile.TileContext(nc) as tc:
        from contextlib import ExitStack
        ctx = ExitStack()
        with ctx:
            consts = ctx.enter_context(tc.tile_pool(name="consts", bufs=1))
            adjp = ctx.enter_context(tc.tile_pool(name="adjp", bufs=1))
            mp = ctx.enter_context(tc.tile_pool(name="mp", bufs=1))
            pp = ctx.enter_context(tc.tile_pool(name="pp", bufs=2))
            outp = ctx.enter_context(tc.tile_pool(name="outp", bufs=1))
            smallp = ctx.enter_context(tc.tile_pool(name="smallp", bufs=4))

            # ---- constants into SBUF ----
            # All big transfers ride the Sync ring in consumption order.
            # The first transfer is one packed block (w | e2v broadcast |
            # first 1024 columns of both x halves) so the z matmuls and
            # field builds start after a single issue+drain; adj q0 rides
            # between x chunk pairs so the first mask TT fires mid-z-phase.
            xpack = consts.tile([128, 3584], dt.bfloat16, tag="xpack")
            nc.sync.dma_start(out=xpack[:], in_=xpack_d[:])
            w0 = xpack[:, 0:O]
            w1 = xpack[:, O:2 * O]
            e2v_bc = xpack[:, 512:1536]
            u12_sb = consts.tile([128, 2 * JT], dt.float32, tag="u12_sb")
            nc.sync.dma_start(out=u12_sb[:], in_=u12_col[:])
            u1_sb = u12_sb[:, 0:JT]
            u2_sb = u12_sb[:, JT:2 * JT]
            adjts = [adjp.tile([128, 4, NI], dt.bfloat16, name=f"adjt{q % 4}",
                               tag=f"adjt{q % 4}")
                     for q in range(NQ)]
            XR = N - NI                       # 3072 remaining x columns
            x0r = consts.tile([128, XR], dt.bfloat16, tag="x0r")
            x1r = consts.tile([128, XR], dt.bfloat16, tag="x1r")
            XC = XR // 3
            for cki in range(3):
                cs = slice(cki * XC, (cki + 1) * XC)
                nc.sync.dma_start(out=x0r[:, cs], in_=xrest_d[0:128, cs])
                nc.sync.dma_start(out=x1r[:, cs], in_=xrest_d[128:256, cs])
                if cki == 0:
                    nc.sync.dma_start(out=adjts[0][:], in_=adjq_view[0])
            for q in range(1, 3):
                nc.sync.dma_start(out=adjts[q][:], in_=adjq_view[q])

            def x_sl(k, nt):
                if nt < 8:
                    base = 1536 if k == 0 else 2560
                    return xpack[:, base + nt * 128:base + (nt + 1) * 128]
                xr = x0r if k == 0 else x1r
                return xr[:, (nt - 8) * 128:(nt - 7) * 128]

            # ---- z phase: z_aug[j, 0:256] = x @ W.T (no bias), col 256 = 1 ----
            z_all = consts.tile([128, JT, O + 1], dt.bfloat16, tag="z_all")
            nc.vector.memset(z_all[:, :, O], 1.0)
            with tc.tile_pool(name="zpsum", bufs=2, space="PSUM") as zpsum:
                for oc in range(4):
                    zp = zpsum.tile([128, 8 * O], dt.float32, name="zp")
                    for t in range(8):
                        nt = oc * 8 + t
                        od = zp[:, t * O:(t + 1) * O]
                        nc.tensor.matmul(od, x_sl(0, nt), w0,
                                         start=True, stop=False)
                        nc.tensor.matmul(od, x_sl(1, nt), w1,
                                         start=False, stop=True)
                    zsrc = zp[:].rearrange("p (t o) -> p t o", t=8)
                    nc.scalar.copy(z_all[:, oc * 8:(oc + 1) * 8, 0:O], zsrc)

            # ---- main loop over quads of 4 j-tiles ----
            accp = ctx.enter_context(tc.tile_pool(name="accp", bufs=1, space="PSUM"))
            accs = [accp.tile([128, O + 1], dt.float32, tag=f"acc{ic}",
                              name=f"acc{ic}")
                    for ic in range(IC)]

            # VectorE queue is strict FIFO, so the emit order IS the V
            # schedule: keep one quad of TS builds ahead of each mask TT so
            # TTs fire as soon as adj lands and TensorE paces the loop.
            m_ts = [mp.tile([128, 4, NI], dt.bfloat16, name=f"m{q % 4}",
                            tag=f"m{q % 4}")
                    for q in range(NQ)]

            def emit_ts(q):
                for k in range(4):
                    jt = q * 4 + k
                    js = slice(jt, jt + 1)
                    # m = (e2v * u2_j) max u1_j  -- fused 2-op TS
                    nc.vector.tensor_scalar(m_ts[q][:, k, :], e2v_bc[:],
                                            u2_sb[:, js], u1_sb[:, js],
                                            op0=ALU.mult, op1=ALU.max)

            emit_ts(0)
            emit_ts(1)
            emit_ts(2)
            for q in range(NQ):
                p_t = pp.tile([128, 4, NI], dt.bfloat16, name="p_t")
                nc.vector.tensor_tensor(p_t[:], m_ts[q][:], adjts[q][:],
                                        op=ALU.mult)
                if q + 3 < NQ:
                    nc.sync.dma_start(out=adjts[q + 3][:],
                                      in_=adjq_view[q + 3])
                if q + 3 < NQ:
                    emit_ts(q + 3)

                for k in range(4):
                    jt = q * 4 + k
                    for ic in range(IC):
                        nc.tensor.matmul(
                            accs[ic][:], p_t[:, k, ic * 128:(ic + 1) * 128],
                            z_all[:, jt, :],
                            start=(jt == 0), stop=(jt == JT - 1))

            # ---- normalize + store (x 1/denominator; bias bW added on host) ----
            # Normalizes alternate between ScalarE and VectorE so the eight
            # chains drain in parallel; a single strided DMA stores all rows.
            o_all = outp.tile([128, IC, O], dt.bfloat16, tag="o_all")
            for ic in range(IC):
                r_t = smallp.tile([128, 1], dt.float32, tag="r", name="r_t")
                nc.vector.reciprocal(r_t[:], accs[ic][:, O:O + 1])
                if ic % 2 == 0:
                    nc.scalar.mul(o_all[:, ic, :], accs[ic][:, 0:O], r_t[:])
                else:
                    nc.vector.tensor_scalar_mul(o_all[:, ic, :],
                                                accs[ic][:, 0:O], r_t[:])
            out_view = out.ap().rearrange("(ic p) o -> p ic o", p=128)
            nc.sync.dma_start(out=out_view[:, 0:4, :], in_=o_all[:, 0:4, :])
            nc.sync.dma_start(out=out_view[:, 4:8, :], in_=o_all[:, 4:8, :])

    nc.compile()
    return nc


def _get_nc():
    if "nc" not in _CACHE:
        _CACHE["nc"] = _build()
    return _CACHE["nc"]


def kernel(x, adjs, W, bW, wai, bai, waj, baj):
    from concourse import bass_utils

    bf16 = ml_dtypes.bfloat16
    x = np.asarray(x, np.float32)
    adjs = np.asarray(adjs, np.float32)
    W = np.asarray(W, np.float32)
    bW = np.asarray(bW, np.float32)
    wai = np.asarray(wai, np.float32)
    bai = np.asarray(bai, np.float32)
    waj = np.asarray(waj, np.float32)
    baj = np.asarray(baj, np.float32)

    # host-folded attention projections (f64 for accuracy)
    u_i = W.astype(np.float64).T @ wai.astype(np.float64).T        # [256,1]
    c_i = float(bW.astype(np.float64) @ wai[0].astype(np.float64)
                + bai.astype(np.float64)[0])
    u_j = W.astype(np.float64).T @ waj.astype(np.float64).T
    c_j = float(bW.astype(np.float64) @ waj[0].astype(np.float64)
                + baj.astype(np.float64)[0])
    ai = (x.astype(np.float64) @ u_i)[:, :, 0] + c_i               # [B,N] f64
    aj = (x.astype(np.float64) @ u_j)[:, :, 0] + c_j

    wc = np.empty((128, 2 * O), bf16)
    wc[:, 0:O] = W.T[0:128, :].astype(bf16)
    wc[:, O:2 * O] = W.T[128:256, :].astype(bf16)
    xT_b, xrest_b, u12_b = [], [], []
    for b in range(B):
        xb = x[b].T.astype(bf16)                       # [256, N]
        xT_b.append(xb)
        xrest_b.append(np.ascontiguousarray(xb[:, NI:]))
        u12 = np.empty((128, 2 * JT), np.float32)
        u12[:, 0:JT] = np.exp(aj[b]).astype(np.float32).reshape(JT, 128).T
        u12[:, JT:] = np.exp(ALPHA * aj[b]).astype(np.float32).reshape(JT, 128).T
        u12_b.append(u12)

    in_maps = []
    for c in range(NCORES):
        b, s = divmod(c, SLABS_PER_B)
        i0 = s * NI
        adjsT_slab = np.ascontiguousarray(adjs[b][i0:i0 + NI, :].T).astype(bf16)
        xpack = np.empty((128, 3584), bf16)
        xpack[:, 0:512] = wc
        xpack[:, 512:1536] = np.exp(
            -0.8 * ai[b, i0:i0 + NI]).astype(bf16)[None, :]
        xpack[:, 1536:2560] = xT_b[b][0:128, 0:NI]
        xpack[:, 2560:3584] = xT_b[b][128:256, 0:NI]
        in_maps.append({
            "adjsT": adjsT_slab,
            "xpack": xpack,
            "xrest": xrest_b[b],
            "u12_col": u12_b[b],
        })

    nc = _get_nc()
    res = bass_utils.run_bass_kernel_spmd(
        nc, in_maps, core_ids=list(range(NCORES)),
        **_CACHE.get("run_kwargs", {}))
    _CACHE["last_results"] = res

    out = np.empty((B, N, O), np.float32)
    for c in range(NCORES):
        b, s = divmod(c, SLABS_PER_B)
        out[b, s * NI:(s + 1) * NI, :] = (
            res.results[c]["out"].astype(np.float32) + bW[None, :])
    return out
